# revision 79
# baseline (speedup 1.0000x reference)
"""DGCNN (4x EdgeConv + FC head) Bass kernel for 8 trn2 NeuronCores — v2.

Sharding: cloud b -> cores {2b, 2b+1}; each core owns 1024 query points
(q0 = (pid % 2) * 1024). Full cloud features exchanged within each pair via
bf16 AllGather after layers 1-3.

Design vs v1 baseline:
- bf16 compute throughout (PE 1 cyc/row incl. transposes; halved DMA bytes).
- Transposed dma_gather delivers neighbor features feature-major: no PE
  transposes in the edge MLP. Gathers raw x_j (dim D), h1 computed directly
  as wbot^T x_j + u_i via PE (u injected with a selector matmul).
- Top-k via composite packing: u32 = (bf16(score) << 16) | idx. Segment Max8
  (8x256) + 3 merge rounds on 64 candidates; no full-width MaxIndex scans.
- -|xj|^2/2 folded into the dist matmul as two bf16 hi/lo rows.
- max over K neighbors via single tensor_reduce per (qh, m) from PSUM.
- PSUM: dist 2 banks + h1 2 + h2acc <=4 = 8, allows cross-tile pipelining.
"""
import numpy as np
import ml_dtypes

import concourse.bass as bass
import concourse.bacc as bacc
import concourse.mybir as mybir
import concourse.tile as tile
from concourse.bass_utils import run_bass_kernel_spmd

B, P, K = 4, 2048, 20
NQ = 1024
N_CORES = 8
NEG = -3.0e38
F32 = mybir.dt.float32
BF16 = mybir.dt.bfloat16
U32 = mybir.dt.uint32
U16 = mybir.dt.uint16
I16 = mybir.dt.int16
AF = mybir.ActivationFunctionType
ALU = mybir.AluOpType
AX = mybir.AxisListType

#          D  Dpad  DH   DO
LCFG = [(3, 128, 64, 64),
        (64, 128, 128, 128),
        (128, 128, 256, 256),
        (256, 256, 512, 512)]
FC1_CHUNKS = [64, 128, 128, 128, 128, 128, 128, 128]  # 960 rows

_CACHED = {}


def cdiv(a, b):
    return (a + b - 1) // b


def _wpack_layout():
    """(key, rows, cols, col_off) chunks of the packed bf16 weight tensor."""
    lay, off = [], 0

    def add(key, r, c):
        nonlocal off
        lay.append((key, r, c, off))
        off += c

    for li, (D, DP, DH, DO) in enumerate(LCFG, start=1):
        for c0 in range(0, D, 128):
            add(f"wdiff{li}", min(128, D - c0), DH)
        for c0 in range(0, DP, 128):
            add(f"wbot{li}", 128, DH)
        for c0 in range(0, DH, 128):
            add(f"wb{li}", min(128, DH - c0), DO)
        add(f"ba{li}", 2, DH)
    for r in FC1_CHUNKS:
        add("fw1", r, 512)
    for _ in range(4):
        add("fw2", 128, 256)
    for _ in range(2):
        add("fw3", 128, 1)
    return lay, off


def _fpack_layout():
    """(key, rows, cols, col_off) chunks of the packed f32 bias tensor."""
    lay, off = [], 0

    def add(key, r, c):
        nonlocal off
        lay.append((key, r, c, off))
        off += c

    for li, (_, _, _, DO) in enumerate(LCFG, start=1):
        for c0 in range(0, DO, 128):
            add(f"bb{li}", min(128, DO - c0), 1)
    for c0 in range(0, 512, 128):
        add("fb1", 128, 1)
    for c0 in range(0, 256, 128):
        add("fb2", 128, 1)
    add("fb3", 1, 1)
    return lay, off


def _build():
    nc = bacc.Bacc("TRN2", target_bir_lowering=False, debug=False,
                   num_devices=N_CORES, num_swdge_queues=4)

    # ---------------- DRAM params ----------------
    xTb1_in = nc.declare_dram_parameter("xTb1", [3, P], BF16, isOutput=False)
    xsb1_in = nc.declare_dram_parameter("xsb1", [128, 16 * 128], BF16, isOutput=False)
    nsq1_in = nc.declare_dram_parameter("nsq1", [2, P], BF16, isOutput=False)
    selI_in = nc.declare_dram_parameter("selI", [16, 320], BF16, isOutput=False)
    wlay, wcols = _wpack_layout()
    flay, fcols = _fpack_layout()
    wpack_in = nc.declare_dram_parameter("wpack", [128, wcols], BF16, isOutput=False)
    fpack_in = nc.declare_dram_parameter("fpack", [128, fcols], F32, isOutput=False)
    y_out = nc.declare_dram_parameter("y", [1, NQ], F32, isOutput=True)

    groups = [[2 * b, 2 * b + 1] for b in range(N_CORES // 2)]

    with tile.TileContext(nc) as tc:
        with tc.tile_pool(name="const", bufs=1) as cp, \
             tc.tile_pool(name="glob", bufs=1) as gp, \
             tc.tile_pool(name="dram", bufs=1, space="DRAM") as dram:

            parity = nc.sync.partition_id()
            parity = nc.sync.scalar_reg_alu(ALU.mod, parity, 2)

            selI = cp.tile([16, 320], BF16, name="selI")
            nc.scalar.dma_start(selI[:], selI_in[:, :])
            ones2 = cp.tile([2, 128], BF16, name="ones2")
            nc.vector.memset(ones2[:], 1.0)
            onescol = cp.tile([128, 1], BF16, name="onescol")
            nc.vector.memset(onescol[:], 1.0)

            # all weights arrive in two packed tensors -> two DMACopies
            # (dozens of small loads would serialize ~30us on the HWDGE)
            wpt = cp.tile([128, wcols], BF16, name="wpt")
            nc.sync.dma_start(wpt[:], wpack_in[:, :])
            fpt = cp.tile([128, fcols], F32, name="fpt")
            nc.sync.dma_start(fpt[:], fpack_in[:, :])
            W = {}
            for key, r, c, off in wlay:
                W.setdefault(key, []).append(wpt[0:r, off:off + c])
            for key, r, c, off in flay:
                W.setdefault(key, []).append(fpt[0:r, off:off + c])
            fw1_tiles = W["fw1"]
            fw2_tiles = W["fw2"]
            fw3_tiles = W["fw3"]
            fbs = {nm: W[nm] for nm in ("fb1", "fb2", "fb3")}

            # persistent double-buffered per-tile structures
            comp = []
            for i in range(2):
                t = gp.tile([128, P], U32, name=f"comp{i}")
                nc.gpsimd.iota(t[:], [[1, P]], base=0, channel_multiplier=0)
                comp.append(t)
            wrapped = []
            for i in range(2):
                t = gp.tile([128, 8 * K], I16, name=f"wrap{i}")
                nc.vector.memset(t[:], 0)
                wrapped.append(t)
            scown = [gp.tile([128, NQ], BF16, name=f"scown{i}") for i in range(8)]
            segtop = [gp.tile([128, 64], F32, name=f"segtop{i}") for i in range(2)]
            top24 = [gp.tile([128, 24], F32, name=f"top24{i}") for i in range(2)]
            idx16 = [gp.tile([128, 24], I16, name=f"idx16{i}") for i in range(2)]
            idx_dram = [dram.tile([128, K], I16, name=f"idxd{i}") for i in range(2)]

            # resident per-layer outputs (feature-major) for the FC head
            xoT = {}
            for li, (_, _, _, DO) in enumerate(LCFG, start=1):
                xoT[li] = [gp.tile([min(128, DO - c0), NQ], BF16,
                                   name=f"xoT{li}_{c0}")
                           for c0 in range(0, DO, 128)]

            ag_in = [[dram.tile([do, NQ // 2], BF16, name=f"agin{li}_{h}")
                      for h in range(2)]
                     for li, (_, _, _, do) in enumerate(LCFG[:3], start=1)]
            ag_out = [[dram.tile([2 * do, NQ // 2], BF16, name=f"agout{li}_{h}")
                       for h in range(2)]
                      for li, (_, _, _, do) in enumerate(LCFG[:3], start=1)]

            xTb = None     # list of [<=128, P] bf16 feature-major chunks
            nsq2 = None    # [2, P] bf16 hi/lo of -0.5|x|^2

            for li, (D, DP, DH, DO) in enumerate(LCFG, start=1):
                NDC = cdiv(D, 128)     # unpadded contract chunks (dist, u)
                NDCP = DP // 128       # padded contract chunks (gather/h1)
                NHC = cdiv(DH, 128)
                NMC = cdiv(DO, 128)
                h2_bufs = 2 if NMC == 1 else 1
                h1_bufs = 2
                ps_bufs = 4 if NMC <= 2 else 2

                with tc.tile_pool(name=f"l{li}", bufs=1) as lp, \
                     tc.tile_pool(name=f"l{li}w", bufs=2) as wkp, \
                     tc.tile_pool(name=f"l{li}ps", bufs=ps_bufs, space="PSUM") as pdist, \
                     tc.tile_pool(name=f"l{li}h1", bufs=h1_bufs, space="PSUM") as ph1, \
                     tc.tile_pool(name=f"l{li}h2", bufs=h2_bufs, space="PSUM") as ph2:

                    # ---- layer inputs, own-first index space ----
                    # own queries occupy candidate columns 0..NQ; the twin
                    # core's half occupies NQ..P. xq = own features (local,
                    # pre-AllGather); xoth = twin half (post-AllGather).
                    if li == 1:
                        xq, xoth = [], []
                        t = lp.tile([3, NQ], BF16, name="xq1")
                        nc.scalar.dma_start(t[:], xTb1_in[:, 0:NQ])
                        xq.append(t)
                        t = lp.tile([3, NQ], BF16, name="xoth1")
                        nc.scalar.dma_start(t[:], xTb1_in[:, NQ:P])
                        xoth.append(t)
                        xsb = lp.tile([128, 16, 128], BF16, name="xsb1")
                        nc.scalar.dma_start(
                            xsb[:].rearrange("p r d -> p (r d)"), xsb1_in[:, :])
                        nsq2 = lp.tile([2, P], BF16, name="nsq1")
                        nc.scalar.dma_start(nsq2[:], nsq1_in[:, :])
                    else:
                        DPREV = LCFG[li - 2][3]
                        xq = xoT[li - 1]  # own features, already feature-major
                        xsb = lp.tile([128, 16, DP], BF16, name=f"xsb{li}")
                        if DPREV < DP:
                            nc.vector.memset(xsb[:, :, DPREV:DP], 0.0)

                    # pre-AG own-half work: u, xsb own ranks, nsq own quarters
                    usb = []
                    for t in range(8):
                        ups = pdist.tile([128, 512], F32, name="ups", tag="dps")
                        tsl = slice(t * 128, (t + 1) * 128)
                        for ci in range(NDC):
                            nc.tensor.matmul(ups[:, :DH], xq[ci][:, tsl],
                                             W[f"wdiff{li}"][ci],
                                             start=(ci == 0), stop=False)
                        nc.tensor.matmul(ups[:, :DH], ones2[:],
                                         W[f"ba{li}"][0], start=False, stop=True)
                        ut = lp.tile([128, DH], BF16, name=f"ust{t}", tag="ust",
                                     bufs=2)
                        nc.scalar.activation(ut[:], ups[:, :DH], AF.Copy)
                        ud = dram.tile([128, DH], BF16, name=f"ud{li}_{t}")
                        nc.sync.dma_start(ud[:, :], ut[:])
                        uq = lp.tile([16, 8, DH], BF16, name=f"usb{t}")
                        nc.sync.dma_start(
                            uq[:], ud[:, :].rearrange("(qh ql) d -> ql qh d", ql=16))
                        usb.append(uq)

                    if li > 1:
                        DPREV = LCFG[li - 2][3]
                        for ci, xt in enumerate(xq):
                            nc.scalar.dma_start_transpose(
                                xsb[:, 0:8, ci * 128:ci * 128 + xt.shape[0]], xt[:])
                        nsq2 = lp.tile([2, P], BF16, name=f"nsq{li}")
                        nsqlo = lp.tile([1, P], BF16, name=f"nsqlo{li}")
                        sqb = lp.tile([128, NQ], BF16, name=f"sqb{li}")

                        def nsq_quarters(src, base):
                            for nb in range(2):
                                nsqps = pdist.tile([128, 512], F32,
                                                   name="nsqps", tag="dps")
                                for ci, xt in enumerate(src):
                                    r = xt.shape[0]
                                    sl = slice(nb * 512, (nb + 1) * 512)
                                    osl = slice(base + nb * 512,
                                                base + (nb + 1) * 512)
                                    nc.vector.tensor_tensor(
                                        sqb[:r, sl], xt[:, sl], xt[:, sl],
                                        op=ALU.mult)
                                    nc.tensor.matmul(
                                        nsqps[0:1, :], onescol[:r, :],
                                        sqb[:r, sl], start=(ci == 0),
                                        stop=(ci == len(src) - 1))
                                nc.scalar.activation(
                                    nsq2[0:1, osl], nsqps[0:1, :],
                                    AF.Copy, scale=-0.5)
                                nc.vector.scalar_tensor_tensor(
                                    nsqlo[0:1, osl], nsqps[0:1, :],
                                    -0.5, nsq2[0:1, osl],
                                    op0=ALU.mult, op1=ALU.subtract)

                        nsq_quarters(xq, 0)
                        nc.sync.dma_start(nsq2[1:2, 0:NQ], nsqlo[0:1, 0:NQ])

                    # phase A: own-half dist for all tiles (pre-AllGather)
                    for t in range(8):
                        tsl = slice(t * 128, (t + 1) * 128)
                        for nb in range(2):
                            dps = pdist.tile([128, 512], F32, name="dpsA", tag="dps")
                            sl = slice(nb * 512, (nb + 1) * 512)
                            for ci in range(NDC):
                                nc.tensor.matmul(dps[:], xq[ci][:, tsl],
                                                 xq[ci][:, sl],
                                                 start=(ci == 0), stop=False)
                            nc.tensor.matmul(dps[:], ones2[:], nsq2[:, sl],
                                             start=False, stop=True)
                            nc.scalar.activation(scown[t][:, sl], dps[:], AF.Copy)

                    if li > 1:
                        DPREV = LCFG[li - 2][3]
                        # post-AG other-half inputs
                        othoff = nc.sync.scalar_reg_alu(ALU.mult, parity, -DPREV)
                        othoff = nc.sync.scalar_reg_alu(ALU.add, othoff, DPREV)
                        xoth = []
                        for c0 in range(0, DPREV, 128):
                            r = min(128, DPREV - c0)
                            rowreg = nc.sync.scalar_reg_alu(ALU.add, othoff, c0)
                            t = lp.tile([r, NQ], BF16, name=f"xoth{li}_{c0}")
                            for h in range(2):
                                nc.sync.dma_start(
                                    t[:, h * 512:(h + 1) * 512],
                                    ag_out[li - 2][h][bass.ds(rowreg, r), :])
                            xoth.append(t)
                        for ci, xt in enumerate(xoth):
                            nc.scalar.dma_start_transpose(
                                xsb[:, 8:16, ci * 128:ci * 128 + xt.shape[0]], xt[:])
                        nsq_quarters(xoth, NQ)
                        nc.sync.dma_start(nsq2[1:2, NQ:P], nsqlo[0:1, NQ:P])

                    macc = [lp.tile([min(128, DO - c0), NQ], BF16,
                                    name=f"macc{li}_{c0}")
                            for c0 in range(0, DO, 128)]

                    if li == 4:
                        h1fc = [lp.tile([128, NQ], BF16, name=f"h1fc{m}")
                                for m in range(4)]
                        # retired phase-A score buffers, exact shape match
                        h2fc = [scown[0], scown[1]]
                        # fc3 output reuses retired score buffers (bitcast)
                        yt_g = [scown[2][0:1, :].bitcast(F32),
                                scown[3][0:1, :].bitcast(F32)]
                        feats = [xoT[1][0], xoT[2][0], xoT[3][0], xoT[3][1],
                                 xoT[4][0], xoT[4][1], xoT[4][2], xoT[4][3]]

                        def emit_fc1(g):
                            gsl = slice(g * 512, (g + 1) * 512)
                            for m in range(4):
                                ps = pdist.tile([128, 512], F32, name="fps",
                                                tag="dps")
                                for ci, ft in enumerate(feats):
                                    nc.tensor.matmul(
                                        ps[:],
                                        fw1_tiles[ci][:, m * 128:(m + 1) * 128],
                                        ft[:, gsl],
                                        start=(ci == 0), stop=(ci == 7))
                                nc.scalar.activation(h1fc[m][:, gsl], ps[:],
                                                     AF.Relu, bias=fbs["fb1"][m])

                        def emit_fc2(g):
                            gsl = slice(g * 512, (g + 1) * 512)
                            for m in range(2):
                                ps = pdist.tile([128, 512], F32, name="fps2",
                                                tag="dps")
                                for ci in range(4):
                                    nc.tensor.matmul(
                                        ps[:],
                                        fw2_tiles[ci][:, m * 128:(m + 1) * 128],
                                        h1fc[ci][:, gsl],
                                        start=(ci == 0), stop=(ci == 3))
                                nc.scalar.activation(h2fc[m][:, gsl], ps[:],
                                                     AF.Relu, bias=fbs["fb2"][m])

                        def emit_fc3(g):
                            gsl = slice(g * 512, (g + 1) * 512)
                            ps = pdist.tile([128, 512], F32, name="fps3",
                                            tag="dps")
                            for ci in range(2):
                                nc.tensor.matmul(ps[0:1, :], fw3_tiles[ci],
                                                 h2fc[ci][:, gsl],
                                                 start=(ci == 0), stop=(ci == 1))
                            nc.scalar.activation(yt_g[g][:, :], ps[0:1, :],
                                                 AF.Sigmoid, bias=fbs["fb3"][0])

                    def emit_xo_half(h):
                        """relu(macc + bb) for column half h -> xoT (+ AG in)."""
                        hsl = slice(h * 512, (h + 1) * 512)
                        for m in range(NMC):
                            mr = min(128, DO - m * 128)
                            nc.scalar.activation(xoT[li][m][:, hsl],
                                                 macc[m][:mr, hsl], AF.Relu,
                                                 bias=W[f"bb{li}"][m])
                            if li < 4:
                                nc.sync.dma_start(
                                    ag_in[li - 1][h][m * 128:m * 128 + mr, :],
                                    xoT[li][m][:, hsl])

                    # ---- main per-tile loop ----
                    for t in range(8):
                        if t == 4:
                            emit_xo_half(0)
                        if t == 5 and li < 4:
                            nc.gpsimd.collective_compute(
                                "AllGather", ALU.bypass, replica_groups=groups,
                                ins=[ag_in[li - 1][0].opt()],
                                outs=[ag_out[li - 1][0].opt()])
                        if t == 7 and li == 4:
                            emit_fc1(0)
                        tb = t % 2
                        tq = t % 2
                        tsl = slice(t * 128, (t + 1) * 128)
                        # other-half dist quarters -> bf16 scores written
                        # straight into the composite's odd u16 lanes
                        cb = comp[tb]
                        cbv = cb[:].bitcast(BF16)[:, 1::2]
                        for nb in range(2):
                            dps = pdist.tile([128, 512], F32, name="dps", tag="dps")
                            sl = slice(NQ + nb * 512, NQ + (nb + 1) * 512)
                            rsl = slice(nb * 512, (nb + 1) * 512)
                            for ci in range(NDC):
                                nc.tensor.matmul(dps[:], xq[ci][:, tsl],
                                                 xoth[ci][:, rsl],
                                                 start=(ci == 0), stop=False)
                            nc.tensor.matmul(dps[:], ones2[:], nsq2[:, sl],
                                             start=False, stop=True)
                            nc.scalar.activation(cbv[:, sl], dps[:], AF.Copy)
                        nc.vector.tensor_copy(cb[:].bitcast(U16)[:, 1::2][:, 0:NQ],
                                              scown[t][:].bitcast(U16))
                        compf = cb[:].bitcast(F32)
                        st = segtop[tq]
                        for s in range(8):
                            nc.vector.max(st[:, s * 8:(s + 1) * 8],
                                          compf[:, s * 256:(s + 1) * 256])
                        t24 = top24[tq]
                        for r in range(3):
                            nc.vector.max(t24[:, 8 * r:8 * r + 8], st[:])
                            if r < 2:
                                nc.vector.match_replace(
                                    st[:], t24[:, 8 * r:8 * r + 8], st[:], NEG)
                        nc.vector.tensor_copy(idx16[tq][:],
                                              t24[:].bitcast(I16)[:, 0::2])
                        # wrap indices: dram bounce + 8-block replication
                        nc.sync.dma_start(idx_dram[tq][:, :], idx16[tq][:, 0:K])
                        wsrc = idx_dram[tq][:, :].rearrange(
                            "(qh ql) k -> ql qh k", ql=16)
                        # only the TX Q7 cpu of queue g reads its 16-partition
                        # block (2g+1); block 0 kept for the interpreter.
                        # Unwritten blocks stay 0 from the one-time memset.
                        for bb in (0, 1, 3, 5, 7):
                            nc.sync.dma_start(
                                wrapped[tq][bb * 16:(bb + 1) * 16, :].rearrange(
                                    "ql (qh k) -> ql qh k", k=K), wsrc)
                        # transposed gathers: 4 chunks x 640 edges (2 qh each)
                        vkc = []
                        for g in range(4):
                            vt = wkp.tile([128, NDCP, 640], BF16,
                                          name=f"vk{g}", tag=f"vk{g}")
                            nc.gpsimd.dma_gather(
                                out_ap=vt[:], in_ap=xsb[:].rearrange("p r d -> p (r d)"),
                                idxs_ap=wrapped[tq][:, g * 40:(g + 1) * 40],
                                num_idxs=640, num_idxs_reg=640, elem_size=DP,
                                transpose=True, queue_num=g,
                                sbuf_tokens_per_rank=128,
                                sbuf_free_dim_per_rank=DP * 2,
                                sbuf_free_dim_pad_per_rank=0,
                                sbuf_byte_offset=0)
                            vkc.append(vt)
                        # edge MLP per qh block (320 edges)
                        for qh in range(8):
                            vt = vkc[qh // 2]
                            off = (qh % 2) * 320
                            h1sb = wkp.tile([128, NHC, 320], BF16,
                                            name="h1sb", tag="h1sb", bufs=4)
                            for hc in range(NHC):
                                hr = min(128, DH - hc * 128)
                                h1ps = ph1.tile([128, 320], F32, name="h1ps",
                                                tag="h1ps")
                                for dc in range(NDCP):
                                    nc.tensor.matmul(
                                        h1ps[:hr, :],
                                        W[f"wbot{li}"][dc][:, hc * 128:hc * 128 + hr],
                                        vt[:, dc, off:off + 320],
                                        start=(dc == 0), stop=False)
                                nc.tensor.matmul(
                                    h1ps[:hr, :],
                                    usb[t][:, qh, hc * 128:hc * 128 + hr],
                                    selI[:], start=False, stop=True)
                                nc.scalar.activation(h1sb[:hr, hc, :],
                                                     h1ps[:hr, :], AF.Relu)
                            for m in range(NMC):
                                mr = min(128, DO - m * 128)
                                h2ps = ph2.tile([128, 320], F32, name="h2ps",
                                                tag=f"h2_{m}")
                                for hc in range(NHC):
                                    hr = min(128, DH - hc * 128)
                                    nc.tensor.matmul(
                                        h2ps[:mr, :],
                                        W[f"wb{li}"][hc][:hr, m * 128:m * 128 + mr],
                                        h1sb[:hr, hc, :],
                                        start=(hc == 0), stop=(hc == NHC - 1))
                                nc.vector.tensor_reduce(
                                    macc[m][:mr, t * 128 + qh * 16:
                                            t * 128 + qh * 16 + 16],
                                    h2ps[:mr, :].rearrange("p (k ql) -> p ql k",
                                                           k=K),
                                    axis=AX.X, op=ALU.max)

                    emit_xo_half(1)
                    if li < 4:
                        nc.gpsimd.collective_compute(
                            "AllGather", ALU.bypass, replica_groups=groups,
                            ins=[ag_in[li - 1][1].opt()],
                            outs=[ag_out[li - 1][1].opt()])
                    if li == 4:
                        emit_fc1(1)
                        emit_fc2(0)
                        emit_fc2(1)
                        emit_fc3(0)
                        emit_fc3(1)
                        nc.sync.dma_start(y_out[:, 0:512], yt_g[0][:, :])
                        nc.sync.dma_start(y_out[:, 512:NQ], yt_g[1][:, :])

    nc.compile()
    return nc


def _bf16(a):
    return np.asarray(a, np.float32).astype(ml_dtypes.bfloat16)


def _hilo(row):
    """f32 row -> [2, N] bf16 (hi, residual)."""
    hi = row.astype(ml_dtypes.bfloat16)
    lo = (row - hi.astype(np.float32)).astype(ml_dtypes.bfloat16)
    return np.stack([hi.astype(np.float32), lo.astype(np.float32)]).astype(
        ml_dtypes.bfloat16)


def kernel(**inputs):
    x = np.asarray(inputs["x"], np.float32)          # [8192, 3]
    if "nc" not in _CACHED:
        _CACHED["nc"] = _build()
    nc = _CACHED["nc"]

    selI = np.zeros((16, 320), np.float32)
    for k in range(K):
        for ql in range(16):
            selI[ql, k * 16 + ql] = 1.0

    full, fullf = {}, {}
    for li, (D, DP, DH, DO) in enumerate(LCFG, start=1):
        wa = np.asarray(inputs[f"w{li}a"], np.float32)
        wtop, wbot = wa[:D], wa[D:]
        full[f"wdiff{li}"] = _bf16(wtop - wbot)
        wbp = np.zeros((DP, DH), np.float32)
        wbp[:D] = wbot
        full[f"wbot{li}"] = _bf16(wbp)
        full[f"ba{li}"] = _hilo(np.asarray(inputs[f"b{li}a"], np.float32))
        full[f"wb{li}"] = _bf16(inputs[f"w{li}b"])
        fullf[f"bb{li}"] = np.asarray(inputs[f"b{li}b"], np.float32)[:, None]
    full["fw1"] = _bf16(inputs["fw1"])
    full["fw2"] = _bf16(inputs["fw2"])
    full["fw3"] = _bf16(inputs["fw3"])
    for nm in ("fb1", "fb2", "fb3"):
        fullf[nm] = np.asarray(inputs[nm], np.float32)[:, None]

    wlay, wcols = _wpack_layout()
    flay, fcols = _fpack_layout()
    wpack = np.zeros((128, wcols), dtype=ml_dtypes.bfloat16)
    cur = {}
    for key, r, c, off in wlay:
        r0 = cur.get(key, 0)
        wpack[0:r, off:off + c] = full[key][r0:r0 + r, :]
        cur[key] = r0 + r
    fpack = np.zeros((128, fcols), np.float32)
    cur = {}
    for key, r, c, off in flay:
        r0 = cur.get(key, 0)
        fpack[0:r, off:off + c] = fullf[key][r0:r0 + r, :]
        cur[key] = r0 + r

    base = {"selI": selI.astype(ml_dtypes.bfloat16),
            "wpack": wpack, "fpack": fpack}

    in_maps = []
    for c in range(N_CORES):
        cloud, half = c // 2, c % 2
        xc = x[cloud * P:(cloud + 1) * P]
        # own-first reorder: this core's 1024 query points come first
        xr = np.concatenate([xc[half * NQ:(half + 1) * NQ],
                             xc[(1 - half) * NQ:(2 - half) * NQ]])
        m = dict(base)
        m["xTb1"] = _bf16(xr.T)
        xp = np.zeros((P, 128), np.float32)
        xp[:, :3] = xr
        m["xsb1"] = _bf16(
            xp.reshape(16, 128, 128).transpose(1, 0, 2).reshape(128, 16 * 128))
        m["nsq1"] = _hilo(-0.5 * (xr * xr).sum(1))
        in_maps.append(m)

    res = run_bass_kernel_spmd(nc, in_maps, core_ids=list(range(N_CORES)))
    out = np.empty((B * P, 1), np.float32)
    for c in range(N_CORES):
        cloud, half = c // 2, c % 2
        out[cloud * P + half * NQ: cloud * P + (half + 1) * NQ, 0] = \
            res.results[c]["y"][0]
    return out


# revision 86
# speedup vs baseline: 1.0018x; 1.0018x over previous
"""DGCNN (4x EdgeConv + FC head) Bass kernel for 8 trn2 NeuronCores — v2.

Sharding: cloud b -> cores {2b, 2b+1}; each core owns 1024 query points
(q0 = (pid % 2) * 1024). Full cloud features exchanged within each pair via
bf16 AllGather after layers 1-3.

Design vs v1 baseline:
- bf16 compute throughout (PE 1 cyc/row incl. transposes; halved DMA bytes).
- Transposed dma_gather delivers neighbor features feature-major: no PE
  transposes in the edge MLP. Gathers raw x_j (dim D), h1 computed directly
  as wbot^T x_j + u_i via PE (u injected with a selector matmul).
- Top-k via composite packing: u32 = (bf16(score) << 16) | idx. Segment Max8
  (8x256) + 3 merge rounds on 64 candidates; no full-width MaxIndex scans.
- -|xj|^2/2 folded into the dist matmul as two bf16 hi/lo rows.
- max over K neighbors via single tensor_reduce per (qh, m) from PSUM.
- PSUM: dist 2 banks + h1 2 + h2acc <=4 = 8, allows cross-tile pipelining.
"""
import numpy as np
import ml_dtypes

import concourse.bass as bass
import concourse.bacc as bacc
import concourse.mybir as mybir
import concourse.tile as tile
from concourse.bass_utils import run_bass_kernel_spmd

B, P, K = 4, 2048, 20
NQ = 1024
N_CORES = 8
NEG = -3.0e38
F32 = mybir.dt.float32
BF16 = mybir.dt.bfloat16
U32 = mybir.dt.uint32
U16 = mybir.dt.uint16
I16 = mybir.dt.int16
AF = mybir.ActivationFunctionType
ALU = mybir.AluOpType
AX = mybir.AxisListType

#          D  Dpad  DH   DO
LCFG = [(3, 128, 64, 64),
        (64, 128, 128, 128),
        (128, 128, 256, 256),
        (256, 256, 512, 512)]
FC1_CHUNKS = [64, 128, 128, 128, 128, 128, 128, 128]  # 960 rows

_CACHED = {}


def cdiv(a, b):
    return (a + b - 1) // b


def _wpack_layout():
    """(key, rows, cols, col_off) chunks of the packed bf16 weight tensor."""
    lay, off = [], 0

    def add(key, r, c):
        nonlocal off
        lay.append((key, r, c, off))
        off += c

    for li, (D, DP, DH, DO) in enumerate(LCFG, start=1):
        for c0 in range(0, D, 128):
            add(f"wdiff{li}", min(128, D - c0), DH)
        for c0 in range(0, DP, 128):
            add(f"wbot{li}", 128, DH)
        for c0 in range(0, DH, 128):
            add(f"wb{li}", min(128, DH - c0), DO)
        add(f"ba{li}", 2, DH)
    for r in FC1_CHUNKS:
        add("fw1", r, 512)
    for _ in range(4):
        add("fw2", 128, 256)
    for _ in range(2):
        add("fw3", 128, 1)
    return lay, off


def _fpack_layout():
    """(key, rows, cols, col_off) chunks of the packed f32 bias tensor."""
    lay, off = [], 0

    def add(key, r, c):
        nonlocal off
        lay.append((key, r, c, off))
        off += c

    for li, (_, _, _, DO) in enumerate(LCFG, start=1):
        for c0 in range(0, DO, 128):
            add(f"bb{li}", min(128, DO - c0), 1)
    for c0 in range(0, 512, 128):
        add("fb1", 128, 1)
    for c0 in range(0, 256, 128):
        add("fb2", 128, 1)
    add("fb3", 1, 1)
    return lay, off


def _build():
    nc = bacc.Bacc("TRN2", target_bir_lowering=False, debug=False,
                   num_devices=N_CORES, num_swdge_queues=4)

    # ---------------- DRAM params ----------------
    xTb1_in = nc.declare_dram_parameter("xTb1", [3, P], BF16, isOutput=False)
    xsb1_in = nc.declare_dram_parameter("xsb1", [128, 16 * 128], BF16, isOutput=False)
    nsq1_in = nc.declare_dram_parameter("nsq1", [2, P], BF16, isOutput=False)
    selI_in = nc.declare_dram_parameter("selI", [16, 320], BF16, isOutput=False)
    wlay, wcols = _wpack_layout()
    flay, fcols = _fpack_layout()
    wpack_in = nc.declare_dram_parameter("wpack", [128, wcols], BF16, isOutput=False)
    fpack_in = nc.declare_dram_parameter("fpack", [128, fcols], F32, isOutput=False)
    y_out = nc.declare_dram_parameter("y", [1, NQ], F32, isOutput=True)

    groups = [[2 * b, 2 * b + 1] for b in range(N_CORES // 2)]

    with tile.TileContext(nc) as tc:
        with tc.tile_pool(name="const", bufs=1) as cp, \
             tc.tile_pool(name="glob", bufs=1) as gp, \
             tc.tile_pool(name="dram", bufs=1, space="DRAM") as dram:

            parity = nc.sync.partition_id()
            parity = nc.sync.scalar_reg_alu(ALU.mod, parity, 2)

            selI = cp.tile([16, 320], BF16, name="selI")
            nc.scalar.dma_start(selI[:], selI_in[:, :])
            ones2 = cp.tile([2, 128], BF16, name="ones2")
            nc.vector.memset(ones2[:], 1.0)
            onescol = cp.tile([128, 1], BF16, name="onescol")
            nc.vector.memset(onescol[:], 1.0)

            # all weights arrive in two packed tensors -> two DMACopies
            # (dozens of small loads would serialize ~30us on the HWDGE)
            wpt = cp.tile([128, wcols], BF16, name="wpt")
            nc.sync.dma_start(wpt[:], wpack_in[:, :])
            fpt = cp.tile([128, fcols], F32, name="fpt")
            nc.sync.dma_start(fpt[:], fpack_in[:, :])
            W = {}
            for key, r, c, off in wlay:
                W.setdefault(key, []).append(wpt[0:r, off:off + c])
            for key, r, c, off in flay:
                W.setdefault(key, []).append(fpt[0:r, off:off + c])
            fw1_tiles = W["fw1"]
            fw2_tiles = W["fw2"]
            fw3_tiles = W["fw3"]
            fbs = {nm: W[nm] for nm in ("fb1", "fb2", "fb3")}

            # persistent double-buffered per-tile structures
            comp = []
            for i in range(2):
                t = gp.tile([128, P], U32, name=f"comp{i}")
                nc.gpsimd.iota(t[:], [[1, P]], base=0, channel_multiplier=0)
                comp.append(t)
            wrapped = []
            for i in range(2):
                t = gp.tile([128, 8 * K], I16, name=f"wrap{i}")
                nc.vector.memset(t[:], 0)
                wrapped.append(t)
            scown = [gp.tile([128, NQ], BF16, name=f"scown{i}") for i in range(8)]
            segtop = [gp.tile([128, 64], F32, name=f"segtop{i}") for i in range(2)]
            top24 = [gp.tile([128, 24], F32, name=f"top24{i}") for i in range(2)]
            idx16 = [gp.tile([128, 24], I16, name=f"idx16{i}") for i in range(2)]
            idx_dram = [dram.tile([128, K], I16, name=f"idxd{i}") for i in range(2)]

            # resident per-layer outputs (feature-major) for the FC head
            xoT = {}
            for li, (_, _, _, DO) in enumerate(LCFG, start=1):
                xoT[li] = [gp.tile([min(128, DO - c0), NQ], BF16,
                                   name=f"xoT{li}_{c0}")
                           for c0 in range(0, DO, 128)]

            ag_in = [[dram.tile([do, NQ // 2], BF16, name=f"agin{li}_{h}")
                      for h in range(2)]
                     for li, (_, _, _, do) in enumerate(LCFG[:3], start=1)]
            ag_out = [[dram.tile([2 * do, NQ // 2], BF16, name=f"agout{li}_{h}")
                       for h in range(2)]
                      for li, (_, _, _, do) in enumerate(LCFG[:3], start=1)]

            xTb = None     # list of [<=128, P] bf16 feature-major chunks
            nsq2 = None    # [2, P] bf16 hi/lo of -0.5|x|^2

            for li, (D, DP, DH, DO) in enumerate(LCFG, start=1):
                NDC = cdiv(D, 128)     # unpadded contract chunks (dist, u)
                NDCP = DP // 128       # padded contract chunks (gather/h1)
                NHC = cdiv(DH, 128)
                NMC = cdiv(DO, 128)
                h2_bufs = 2 if NMC == 1 else 1
                h1_bufs = 2
                ps_bufs = 4 if NMC <= 2 else 2

                with tc.tile_pool(name=f"l{li}", bufs=1) as lp, \
                     tc.tile_pool(name=f"l{li}w", bufs=2) as wkp, \
                     tc.tile_pool(name=f"l{li}ps", bufs=ps_bufs, space="PSUM") as pdist, \
                     tc.tile_pool(name=f"l{li}h1", bufs=h1_bufs, space="PSUM") as ph1, \
                     tc.tile_pool(name=f"l{li}h2", bufs=h2_bufs, space="PSUM") as ph2:

                    # ---- layer inputs, own-first index space ----
                    # own queries occupy candidate columns 0..NQ; the twin
                    # core's half occupies NQ..P. xq = own features (local,
                    # pre-AllGather); xoth = twin half (post-AllGather).
                    if li == 1:
                        xq, xoth = [], []
                        t = lp.tile([3, NQ], BF16, name="xq1")
                        nc.scalar.dma_start(t[:], xTb1_in[:, 0:NQ])
                        xq.append(t)
                        t = lp.tile([3, NQ], BF16, name="xoth1")
                        nc.scalar.dma_start(t[:], xTb1_in[:, NQ:P])
                        xoth.append(t)
                        xsb = lp.tile([128, 16, 128], BF16, name="xsb1")
                        nc.scalar.dma_start(
                            xsb[:].rearrange("p r d -> p (r d)"), xsb1_in[:, :])
                        nsq2 = lp.tile([2, P], BF16, name="nsq1")
                        nc.scalar.dma_start(nsq2[:], nsq1_in[:, :])
                    else:
                        DPREV = LCFG[li - 2][3]
                        xq = xoT[li - 1]  # own features, already feature-major
                        xsb = lp.tile([128, 16, DP], BF16, name=f"xsb{li}")
                        if DPREV < DP:
                            nc.vector.memset(xsb[:, :, DPREV:DP], 0.0)

                    # pre-AG own-half work: u, xsb own ranks, nsq own quarters
                    usb = []
                    for t in range(8):
                        ups = pdist.tile([128, 512], F32, name="ups", tag="dps")
                        tsl = slice(t * 128, (t + 1) * 128)
                        for ci in range(NDC):
                            nc.tensor.matmul(ups[:, :DH], xq[ci][:, tsl],
                                             W[f"wdiff{li}"][ci],
                                             start=(ci == 0), stop=False)
                        nc.tensor.matmul(ups[:, :DH], ones2[:],
                                         W[f"ba{li}"][0], start=False, stop=True)
                        ut = lp.tile([128, DH], BF16, name=f"ust{t}", tag="ust",
                                     bufs=2)
                        nc.scalar.activation(ut[:], ups[:, :DH], AF.Copy)
                        ud = dram.tile([128, DH], BF16, name=f"ud{li}_{t}")
                        nc.sync.dma_start(ud[:, :], ut[:])
                        uq = lp.tile([16, 8, DH], BF16, name=f"usb{t}")
                        nc.sync.dma_start(
                            uq[:], ud[:, :].rearrange("(qh ql) d -> ql qh d", ql=16))
                        usb.append(uq)

                    if li > 1:
                        DPREV = LCFG[li - 2][3]
                        for ci, xt in enumerate(xq):
                            nc.scalar.dma_start_transpose(
                                xsb[:, 0:8, ci * 128:ci * 128 + xt.shape[0]], xt[:])
                        nsq2 = lp.tile([2, P], BF16, name=f"nsq{li}")
                        nsqlo = lp.tile([1, P], BF16, name=f"nsqlo{li}")
                        sqb = lp.tile([128, NQ], BF16, name=f"sqb{li}")

                        def nsq_quarters(src, base):
                            for nb in range(2):
                                nsqps = pdist.tile([128, 512], F32,
                                                   name="nsqps", tag="dps")
                                for ci, xt in enumerate(src):
                                    r = xt.shape[0]
                                    sl = slice(nb * 512, (nb + 1) * 512)
                                    osl = slice(base + nb * 512,
                                                base + (nb + 1) * 512)
                                    nc.vector.tensor_tensor(
                                        sqb[:r, sl], xt[:, sl], xt[:, sl],
                                        op=ALU.mult)
                                    nc.tensor.matmul(
                                        nsqps[0:1, :], onescol[:r, :],
                                        sqb[:r, sl], start=(ci == 0),
                                        stop=(ci == len(src) - 1))
                                nc.scalar.activation(
                                    nsq2[0:1, osl], nsqps[0:1, :],
                                    AF.Copy, scale=-0.5)
                                nc.vector.scalar_tensor_tensor(
                                    nsqlo[0:1, osl], nsqps[0:1, :],
                                    -0.5, nsq2[0:1, osl],
                                    op0=ALU.mult, op1=ALU.subtract)

                        nsq_quarters(xq, 0)
                        nc.sync.dma_start(nsq2[1:2, 0:NQ], nsqlo[0:1, 0:NQ])

                    # phase A: own-half dist for all tiles (pre-AllGather)
                    for t in range(8):
                        tsl = slice(t * 128, (t + 1) * 128)
                        for nb in range(2):
                            dps = pdist.tile([128, 512], F32, name="dpsA", tag="dps")
                            sl = slice(nb * 512, (nb + 1) * 512)
                            for ci in range(NDC):
                                nc.tensor.matmul(dps[:], xq[ci][:, tsl],
                                                 xq[ci][:, sl],
                                                 start=(ci == 0), stop=False)
                            nc.tensor.matmul(dps[:], ones2[:], nsq2[:, sl],
                                             start=False, stop=True)
                            nc.scalar.activation(scown[t][:, sl], dps[:], AF.Copy)

                    if li > 1:
                        DPREV = LCFG[li - 2][3]
                        # post-AG other-half inputs
                        othoff = nc.sync.scalar_reg_alu(ALU.mult, parity, -DPREV)
                        othoff = nc.sync.scalar_reg_alu(ALU.add, othoff, DPREV)
                        xoth = []
                        for c0 in range(0, DPREV, 128):
                            r = min(128, DPREV - c0)
                            rowreg = nc.sync.scalar_reg_alu(ALU.add, othoff, c0)
                            t = lp.tile([r, NQ], BF16, name=f"xoth{li}_{c0}")
                            for h in range(2):
                                nc.sync.dma_start(
                                    t[:, h * 512:(h + 1) * 512],
                                    ag_out[li - 2][h][bass.ds(rowreg, r), :])
                            xoth.append(t)
                        for ci, xt in enumerate(xoth):
                            nc.scalar.dma_start_transpose(
                                xsb[:, 8:16, ci * 128:ci * 128 + xt.shape[0]], xt[:])
                        nsq_quarters(xoth, NQ)
                        nc.sync.dma_start(nsq2[1:2, NQ:P], nsqlo[0:1, NQ:P])

                    macc = [lp.tile([min(128, DO - c0), NQ], BF16,
                                    name=f"macc{li}_{c0}")
                            for c0 in range(0, DO, 128)]

                    if li == 4:
                        h1fc = [lp.tile([128, NQ], BF16, name=f"h1fc{m}")
                                for m in range(4)]
                        # retired phase-A score buffers, exact shape match
                        h2fc = [scown[0], scown[1]]
                        # fc3 output reuses retired score buffers (bitcast)
                        yt_g = [scown[2][0:1, :].bitcast(F32),
                                scown[3][0:1, :].bitcast(F32)]
                        feats = [xoT[1][0], xoT[2][0], xoT[3][0], xoT[3][1],
                                 xoT[4][0], xoT[4][1], xoT[4][2], xoT[4][3]]

                        def emit_fc1(g):
                            gsl = slice(g * 512, (g + 1) * 512)
                            for m in range(4):
                                ps = pdist.tile([128, 512], F32, name="fps",
                                                tag="dps")
                                for ci, ft in enumerate(feats):
                                    nc.tensor.matmul(
                                        ps[:],
                                        fw1_tiles[ci][:, m * 128:(m + 1) * 128],
                                        ft[:, gsl],
                                        start=(ci == 0), stop=(ci == 7))
                                nc.scalar.activation(h1fc[m][:, gsl], ps[:],
                                                     AF.Relu, bias=fbs["fb1"][m])

                        def emit_fc2(g):
                            gsl = slice(g * 512, (g + 1) * 512)
                            for m in range(2):
                                ps = pdist.tile([128, 512], F32, name="fps2",
                                                tag="dps")
                                for ci in range(4):
                                    nc.tensor.matmul(
                                        ps[:],
                                        fw2_tiles[ci][:, m * 128:(m + 1) * 128],
                                        h1fc[ci][:, gsl],
                                        start=(ci == 0), stop=(ci == 3))
                                nc.scalar.activation(h2fc[m][:, gsl], ps[:],
                                                     AF.Relu, bias=fbs["fb2"][m])

                        def emit_fc3(g):
                            gsl = slice(g * 512, (g + 1) * 512)
                            ps = pdist.tile([128, 512], F32, name="fps3",
                                            tag="dps")
                            for ci in range(2):
                                nc.tensor.matmul(ps[0:1, :], fw3_tiles[ci],
                                                 h2fc[ci][:, gsl],
                                                 start=(ci == 0), stop=(ci == 1))
                            nc.scalar.activation(yt_g[g][:, :], ps[0:1, :],
                                                 AF.Sigmoid, bias=fbs["fb3"][0])

                    def emit_xo_half(h):
                        """relu(macc + bb) for column half h -> xoT (+ AG in)."""
                        hsl = slice(h * 512, (h + 1) * 512)
                        for m in range(NMC):
                            mr = min(128, DO - m * 128)
                            nc.scalar.activation(xoT[li][m][:, hsl],
                                                 macc[m][:mr, hsl], AF.Relu,
                                                 bias=W[f"bb{li}"][m])
                            if li < 4:
                                nc.sync.dma_start(
                                    ag_in[li - 1][h][m * 128:m * 128 + mr, :],
                                    xoT[li][m][:, hsl])

                    # ---- main per-tile loop ----
                    for t in range(8):
                        if t == 4:
                            emit_xo_half(0)
                        if t == 5 and li < 4:
                            nc.gpsimd.collective_compute(
                                "AllGather", ALU.bypass, replica_groups=groups,
                                ins=[ag_in[li - 1][0].opt()],
                                outs=[ag_out[li - 1][0].opt()])
                        if t == 7 and li == 4:
                            emit_fc1(0)
                        tb = t % 2
                        tq = t % 2
                        tsl = slice(t * 128, (t + 1) * 128)
                        # other-half dist quarters -> bf16 scores written
                        # straight into the composite's odd u16 lanes
                        cb = comp[tb]
                        cbv = cb[:].bitcast(BF16)[:, 1::2]
                        for nb in range(2):
                            dps = pdist.tile([128, 512], F32, name="dps", tag="dps")
                            sl = slice(NQ + nb * 512, NQ + (nb + 1) * 512)
                            rsl = slice(nb * 512, (nb + 1) * 512)
                            for ci in range(NDC):
                                nc.tensor.matmul(dps[:], xq[ci][:, tsl],
                                                 xoth[ci][:, rsl],
                                                 start=(ci == 0), stop=False)
                            nc.tensor.matmul(dps[:], ones2[:], nsq2[:, sl],
                                             start=False, stop=True)
                            nc.scalar.activation(cbv[:, sl], dps[:], AF.Copy)
                        nc.vector.tensor_copy(cb[:].bitcast(U16)[:, 1::2][:, 0:NQ],
                                              scown[t][:].bitcast(U16))
                        compf = cb[:].bitcast(F32)
                        st = segtop[tq]
                        for s in range(8):
                            nc.vector.max(st[:, s * 8:(s + 1) * 8],
                                          compf[:, s * 256:(s + 1) * 256])
                        t24 = top24[tq]
                        for r in range(3):
                            nc.vector.max(t24[:, 8 * r:8 * r + 8], st[:])
                            if r < 2:
                                nc.vector.match_replace(
                                    st[:], t24[:, 8 * r:8 * r + 8], st[:], NEG)
                        nc.vector.tensor_copy(idx16[tq][:],
                                              t24[:].bitcast(I16)[:, 0::2])
                        # wrap indices: dram bounce + 8-block replication
                        nc.sync.dma_start(idx_dram[tq][:, :], idx16[tq][:, 0:K])
                        wsrc = idx_dram[tq][:, :].rearrange(
                            "(qh ql) k -> ql qh k", ql=16)
                        # only the TX Q7 cpu of queue g reads its 16-partition
                        # block (2g+1); block 0 kept for the interpreter.
                        # Unwritten blocks stay 0 from the one-time memset.
                        for bb in (0, 1, 3, 5, 7):
                            nc.sync.dma_start(
                                wrapped[tq][bb * 16:(bb + 1) * 16, :].rearrange(
                                    "ql (qh k) -> ql qh k", k=K), wsrc)
                        # transposed gathers: 4 chunks x 640 edges (2 qh each)
                        vkc = []
                        for g in range(4):
                            vt = wkp.tile([128, NDCP, 640], BF16,
                                          name=f"vk{g}", tag=f"vk{g}")
                            nc.gpsimd.dma_gather(
                                out_ap=vt[:], in_ap=xsb[:].rearrange("p r d -> p (r d)"),
                                idxs_ap=wrapped[tq][:, g * 40:(g + 1) * 40],
                                num_idxs=640, num_idxs_reg=640, elem_size=DP,
                                transpose=True, queue_num=g,
                                sbuf_tokens_per_rank=128,
                                sbuf_free_dim_per_rank=DP * 2,
                                sbuf_free_dim_pad_per_rank=0,
                                sbuf_byte_offset=0)
                            vkc.append(vt)
                        # edge MLP per qh block (320 edges)
                        for qh in range(8):
                            vt = vkc[qh // 2]
                            off = (qh % 2) * 320
                            h1sb = wkp.tile([128, NHC, 320], BF16,
                                            name="h1sb", tag="h1sb", bufs=4)
                            for hc in range(NHC):
                                hr = min(128, DH - hc * 128)
                                h1ps = ph1.tile([128, 320], F32, name="h1ps",
                                                tag="h1ps")
                                for dc in range(NDCP):
                                    nc.tensor.matmul(
                                        h1ps[:hr, :],
                                        W[f"wbot{li}"][dc][:, hc * 128:hc * 128 + hr],
                                        vt[:, dc, off:off + 320],
                                        start=(dc == 0), stop=False)
                                nc.tensor.matmul(
                                    h1ps[:hr, :],
                                    usb[t][:, qh, hc * 128:hc * 128 + hr],
                                    selI[:], start=False, stop=True)
                                nc.scalar.activation(h1sb[:hr, hc, :],
                                                     h1ps[:hr, :], AF.Relu)
                            for m in range(NMC):
                                mr = min(128, DO - m * 128)
                                h2ps = ph2.tile([128, 320], F32, name="h2ps",
                                                tag=f"h2_{m}")
                                for hc in range(NHC):
                                    hr = min(128, DH - hc * 128)
                                    nc.tensor.matmul(
                                        h2ps[:mr, :],
                                        W[f"wb{li}"][hc][:hr, m * 128:m * 128 + mr],
                                        h1sb[:hr, hc, :],
                                        start=(hc == 0), stop=(hc == NHC - 1))
                                nc.vector.tensor_reduce(
                                    macc[m][:mr, t * 128 + qh * 16:
                                            t * 128 + qh * 16 + 16],
                                    h2ps[:mr, :].rearrange("p (k ql) -> p ql k",
                                                           k=K),
                                    axis=AX.X, op=ALU.max)

                    emit_xo_half(1)
                    if li < 4:
                        nc.gpsimd.collective_compute(
                            "AllGather", ALU.bypass, replica_groups=groups,
                            ins=[ag_in[li - 1][1].opt()],
                            outs=[ag_out[li - 1][1].opt()])
                    if li == 4:
                        emit_fc2(0)
                        emit_fc1(1)
                        emit_fc3(0)
                        emit_fc2(1)
                        emit_fc3(1)
                        nc.sync.dma_start(y_out[:, 0:512], yt_g[0][:, :])
                        nc.sync.dma_start(y_out[:, 512:NQ], yt_g[1][:, :])

    nc.compile()
    return nc


def _bf16(a):
    return np.asarray(a, np.float32).astype(ml_dtypes.bfloat16)


def _hilo(row):
    """f32 row -> [2, N] bf16 (hi, residual)."""
    hi = row.astype(ml_dtypes.bfloat16)
    lo = (row - hi.astype(np.float32)).astype(ml_dtypes.bfloat16)
    return np.stack([hi.astype(np.float32), lo.astype(np.float32)]).astype(
        ml_dtypes.bfloat16)


def kernel(**inputs):
    x = np.asarray(inputs["x"], np.float32)          # [8192, 3]
    if "nc" not in _CACHED:
        _CACHED["nc"] = _build()
    nc = _CACHED["nc"]

    selI = np.zeros((16, 320), np.float32)
    for k in range(K):
        for ql in range(16):
            selI[ql, k * 16 + ql] = 1.0

    full, fullf = {}, {}
    for li, (D, DP, DH, DO) in enumerate(LCFG, start=1):
        wa = np.asarray(inputs[f"w{li}a"], np.float32)
        wtop, wbot = wa[:D], wa[D:]
        full[f"wdiff{li}"] = _bf16(wtop - wbot)
        wbp = np.zeros((DP, DH), np.float32)
        wbp[:D] = wbot
        full[f"wbot{li}"] = _bf16(wbp)
        full[f"ba{li}"] = _hilo(np.asarray(inputs[f"b{li}a"], np.float32))
        full[f"wb{li}"] = _bf16(inputs[f"w{li}b"])
        fullf[f"bb{li}"] = np.asarray(inputs[f"b{li}b"], np.float32)[:, None]
    full["fw1"] = _bf16(inputs["fw1"])
    full["fw2"] = _bf16(inputs["fw2"])
    full["fw3"] = _bf16(inputs["fw3"])
    for nm in ("fb1", "fb2", "fb3"):
        fullf[nm] = np.asarray(inputs[nm], np.float32)[:, None]

    wlay, wcols = _wpack_layout()
    flay, fcols = _fpack_layout()
    wpack = np.zeros((128, wcols), dtype=ml_dtypes.bfloat16)
    cur = {}
    for key, r, c, off in wlay:
        r0 = cur.get(key, 0)
        wpack[0:r, off:off + c] = full[key][r0:r0 + r, :]
        cur[key] = r0 + r
    fpack = np.zeros((128, fcols), np.float32)
    cur = {}
    for key, r, c, off in flay:
        r0 = cur.get(key, 0)
        fpack[0:r, off:off + c] = fullf[key][r0:r0 + r, :]
        cur[key] = r0 + r

    base = {"selI": selI.astype(ml_dtypes.bfloat16),
            "wpack": wpack, "fpack": fpack}

    in_maps = []
    for c in range(N_CORES):
        cloud, half = c // 2, c % 2
        xc = x[cloud * P:(cloud + 1) * P]
        # own-first reorder: this core's 1024 query points come first
        xr = np.concatenate([xc[half * NQ:(half + 1) * NQ],
                             xc[(1 - half) * NQ:(2 - half) * NQ]])
        m = dict(base)
        m["xTb1"] = _bf16(xr.T)
        xp = np.zeros((P, 128), np.float32)
        xp[:, :3] = xr
        m["xsb1"] = _bf16(
            xp.reshape(16, 128, 128).transpose(1, 0, 2).reshape(128, 16 * 128))
        m["nsq1"] = _hilo(-0.5 * (xr * xr).sum(1))
        in_maps.append(m)

    res = run_bass_kernel_spmd(nc, in_maps, core_ids=list(range(N_CORES)))
    out = np.empty((B * P, 1), np.float32)
    for c in range(N_CORES):
        cloud, half = c // 2, c % 2
        out[cloud * P + half * NQ: cloud * P + (half + 1) * NQ, 0] = \
            res.results[c]["y"][0]
    return out
